# revision 1
# baseline (speedup 1.0000x reference)
"""Trainium2 Bass kernel for nn_ConvClassifier (predictive-coding network, 20 steps).

Formulation: all up2/sumpool2/convT/conv pairs are folded into 9-tap
"parity-combined" 3x3 convolutions over parity planes (validated vs the JAX
reference in a numpy prototype, rel err ~4e-6):
  x1 is stored as 128 = (4 parity x 32 ch) partition planes of 12x12.
  p1 = convT(up2(relu(x2)), W2)        -> 9-tap conv, K=64,  M=128
  sumpool2(conv(e1, W2))               -> 9-tap conv, K=128, M=64
  p2 / sumpool2(conv(e2, W3))          -> per-parity 9-tap convs K=64, M=64
  p0 = convT(relu(x1), W1)             -> 9-tap conv, K=128, M=4 (parities)
  conv(e0, W1)                         -> 9-tap conv, K=4,   M=128
  FC layers are plain matmuls with batch on the moving dim.

Data-parallel over 8 cores (256 samples each), 8 chunks of 32 samples per
core; each chunk runs all 20 steps entirely out of SBUF (states fp32,
conv matmuls in float32r), then writes its x5 back.
"""

import sys

sys.path.insert(0, "/opt/trn_rl_repo")

import numpy as np

import concourse.bass as bass
import concourse.tile as tile
import concourse.tile_sem_assignment as _tsa
from concourse import mybir
from concourse.alu_op_type import AluOpType as Op

# Pin all HWDGE DMAs to fewer semaphore lanes: this toolchain's walrus rejects
# instructions whose wait list spans many procs ("Too many sync wait commands"),
# and 8-way DMA-queue spreading is the main contributor.
_tsa.NUM_HWDGE_SEMS = 1

F32 = mybir.dt.float32
F32R = mybir.dt.float32  # fp32r: 2.4e-2 err over 20 steps; ship exact fp32
RELU = mybir.ActivationFunctionType.Relu

B = 2048
NCORES = 8
BC = B // NCORES          # 256 samples per core
BT = 32                   # samples per chunk
NCHUNK = BC // BT         # 8
STEPS = 20
GAMMA = 0.1

# sample-chunk sizes for the conv N-chunking (PSUM bank is 512 fp32).
# Matmul outputs are computed on width-padded rows (flat overlapping windows,
# since MM APs allow only 2 free dims); trailing junk columns are skipped by
# the PSUM consumers.
P0NB = 2   # N = 2*14*16 = 448
G0NB = 3   # N = 3*12*14 = 504
P1NB = 3   # N = 504
A2NB = 4   # N = 4*10*12 = 480
P2NB = 14  # N = 14*5*7 = 490


def _chunks(total, per):
    out = []
    b0 = 0
    while b0 < total:
        nb = min(per, total - b0)
        out.append((b0, nb))
        b0 += nb
    return out


# ---------------- combined-kernel builders (validated in proto.py) ----------------

def _build_Wc_up(W):
    I, O = W.shape[:2]
    Wf = W[:, :, ::-1, ::-1].transpose(1, 0, 2, 3)
    Wc = np.zeros((2, 2, O, I, 3, 3), np.float32)
    for qy in range(2):
        for qx in range(2):
            for a in range(5):
                dr = (qy + a) // 2 - 2
                for c in range(5):
                    dc = (qx + c) // 2 - 2
                    Wc[qy, qx, :, :, dr + 2, dc + 2] += Wf[:, :, a, c]
    return Wc.reshape(4 * O, I, 3, 3)


def _build_Wa_down(W):
    J, O = W.shape[:2]
    Wa = np.zeros((2, 2, O, J, 3, 3), np.float32)
    for py in range(2):
        for a in range(5):
            qy = (py + a) % 2
            dr = (py + a) // 2
            for px in range(2):
                for c in range(5):
                    qx = (px + c) % 2
                    dc = (px + c) // 2
                    Wa[qy, qx, :, :, dr, dc] += W[:, :, a, c].T
    return Wa.reshape(4 * O, J, 3, 3)


def _build_Wp0(W1):
    I = W1.shape[0]
    Wf = W1[:, :, ::-1, ::-1].transpose(1, 0, 2, 3)[0]
    Wp0 = np.zeros((2, 2, 2, 2, I, 3, 3), np.float32)
    for qy in range(2):
        for a in range(5):
            py = (qy + a) % 2
            dr = (qy + a) // 2 - 2
            for qx in range(2):
                for c in range(5):
                    px = (qx + c) % 2
                    dc = (qx + c) // 2 - 2
                    Wp0[qy, qx, py, px, :, dr + 2, dc + 2] += Wf[:, a, c]
    return Wp0.reshape(4, 4 * I, 3, 3)


def _build_Wg0(W1):
    O = W1.shape[0]
    Wg = np.zeros((2, 2, O, 2, 2, 3, 3), np.float32)
    for py in range(2):
        for a in range(5):
            qy = (py + a) % 2
            dr = (py + a) // 2
            for px in range(2):
                for c in range(5):
                    qx = (px + c) % 2
                    dc = (px + c) // 2
                    Wg[py, px, :, qy, qx, dr, dc] += W1[:, 0, a, c]
    return Wg.reshape(4 * O, 4, 3, 3)


def _to_parity(x):
    Bn, C, H2, W2 = x.shape
    H, W = H2 // 2, W2 // 2
    y = x.reshape(Bn, C, H, 2, W, 2).transpose(0, 3, 5, 1, 2, 4)
    return y.reshape(Bn, 4 * C, H, W)


def make_weight_arrays(W1, b1, W2, b2, W3, b3, W4, b4, W5, b5, bt):
    Wc2 = _build_Wc_up(W2)        # [128, 64, 3, 3]
    Wa2 = _build_Wa_down(W2)      # [128, 64, 3, 3] (K, M)
    Wc3 = _build_Wc_up(W3)        # [256, 64, 3, 3]
    Wa3 = _build_Wa_down(W3)      # [256, 64, 3, 3] (K, M)
    Wp0 = _build_Wp0(W1)          # [4, 128, 3, 3]
    Wg0 = _build_Wg0(W1)          # [128, 4, 3, 3]
    w = {}
    w["wt_p0"] = np.ascontiguousarray(
        Wp0.transpose(1, 2, 3, 0).reshape(128, 36), np.float32)
    g0w = -Wg0.transpose(1, 2, 3, 0).reshape(4, 9 * 128)
    g0r = np.zeros((128, 9 * 128), np.float32)
    for q in range(4):
        g0r[32 * q:32 * q + 4] = g0w
    w["wt_g0"] = g0r
    p1w = Wc2.transpose(1, 2, 3, 0).reshape(64, 9 * 128)
    p1r = np.zeros((128, 9 * 128), np.float32)
    for t in range(9):  # taps 0-4 on PE rows 0-63, taps 5-8 on rows 64-127
        r0 = 0 if t < 5 else 64
        p1r[r0:r0 + 64, t * 128:(t + 1) * 128] = p1w[:, t * 128:(t + 1) * 128]
    w["wt_p1"] = p1r
    w["wt_a2"] = np.ascontiguousarray(
        -Wa2.transpose(0, 2, 3, 1).reshape(128, 9 * 64), np.float32)
    p2w = Wc3.reshape(4, 64, 64, 3, 3).transpose(2, 0, 3, 4, 1).reshape(
        64, 4 * 9 * 64)
    p2r = np.zeros((128, 4 * 9 * 64), np.float32)
    for pq in range(4):  # parities 0,1 on rows 0-63, parities 2,3 on 64-127
        r0 = 0 if pq < 2 else 64
        p2r[r0:r0 + 64, pq * 576:(pq + 1) * 576] = p2w[:, pq * 576:(pq + 1) * 576]
    w["wt_p2"] = p2r
    a3w = (-Wa3).reshape(4, 64, 64, 3, 3).transpose(1, 0, 3, 4, 2).reshape(
        64, 4 * 9 * 64)
    a3r = np.zeros((128, 4 * 9 * 64), np.float32)
    for pq in range(4):
        r0 = 0 if pq < 2 else 64
        a3r[r0:r0 + 64, pq * 576:(pq + 1) * 576] = a3w[:, pq * 576:(pq + 1) * 576]
    w["wt_a3"] = a3r
    w["wt_p3"] = np.ascontiguousarray(
        W4.reshape(64, 9, 128).transpose(2, 1, 0).reshape(128, 9 * 64), np.float32)
    w["wt_a4"] = np.ascontiguousarray(
        (-W4).reshape(64, 9 * 128), np.float32)
    w["wt_p4"] = np.ascontiguousarray(W5.T, np.float32)       # [10, 128]
    w["wt_a5"] = np.ascontiguousarray(-W5, np.float32)        # [128, 10]
    w["b1r"] = np.full((128, 1), float(b1[0]), np.float32)
    w["b2r"] = np.tile(b2, 4).reshape(128, 1).astype(np.float32)
    w["b3r"] = b3.reshape(64, 1).astype(np.float32)
    w["b5r"] = b5.reshape(128, 1).astype(np.float32)
    w["b4big"] = np.ascontiguousarray(
        np.broadcast_to(b4.reshape(64, 1, 9), (64, bt, 9)).reshape(64, bt * 9),
        np.float32)
    return w


def make_data_arrays(obs, x1, x2, x3, x4, x5):
    """Per-core data slices -> kernel DRAM layouts."""
    bc = obs.shape[0]
    d = {}
    x1p = _to_parity(0.01 * x1)                                  # [bc,128,12,12]
    d["x1p"] = np.ascontiguousarray(
        x1p.transpose(1, 0, 2, 3).reshape(128, bc * 144), np.float32)
    obsp = _to_parity(obs)                                       # [bc,4,14,14]
    # quarter-split layout: row q*4+parity holds quarter-q samples, so p0's
    # four concurrent col-strip tiles and g0's row-strip tiles stay aligned
    oq = obsp.reshape(bc // 32, 4, 8, 4, 196).transpose(1, 3, 0, 2, 4)
    d["obsp"] = np.ascontiguousarray(oq.reshape(16, bc * 49), np.float32)
    d["x2f"] = np.ascontiguousarray(
        (0.01 * x2).transpose(1, 0, 2, 3).reshape(64, bc * 100), np.float32)
    d["x3f"] = np.ascontiguousarray(
        (0.01 * x3).transpose(1, 0, 2, 3).reshape(64, bc * 9), np.float32)
    d["x4t"] = np.ascontiguousarray((0.01 * x4).T, np.float32)   # [128, bc]
    d["x5t"] = np.ascontiguousarray((0.01 * x5).T, np.float32)   # [10, bc]
    return d


# ---------------- Bass kernel ----------------

def _split_multiwait_instructions(nc):
    """This toolchain's walrus accepts at most ONE semaphore wait per
    instruction ("Too many sync wait commands"). Hoist extra waits onto
    same-engine NOPs inserted immediately before the instruction — identical
    semantics, since the NOP occupies the same position in the engine's
    program order."""
    cur = nc.cur_bb.bb
    for blk in nc.m.functions[0].blocks:
        insts = list(blk.instructions)
        changed = False
        out = []
        for inst in insts:
            si = inst.sync_info
            if si is not None and si.on_wait is not None and len(si.on_wait) > 1:
                waits = list(si.on_wait)
                for w in waits[:-1]:
                    nop = nc.engines[inst.engine].nop().ins
                    cl = cur.instructions
                    assert cl[-1] is nop
                    cl.pop()
                    cur.instructions = cl
                    nop.sync_info = mybir.SyncInfo(on_wait=[w], on_update=[])
                    out.append(nop)
                inst.sync_info = mybir.SyncInfo(
                    on_wait=[waits[-1]], on_update=list(si.on_update or []))
                changed = True
            out.append(inst)
        if changed:
            blk.instructions = out


def build_bass(bt=BT, nchunk=NCHUNK, steps=STEPS, bc=None, unroll_steps=False):
    if bc is None:
        bc = bt * nchunk
    nc = bass.Bass("TRN2", target_bir_lowering=False, debug=False,
                   num_devices=NCORES, dynamic_dma_scratch_size=1024)

    dram = {}
    for name, shape in [
        ("x1p", [128, bc * 144]), ("obsp", [16, bc * 49]),
        ("x2f", [64, bc * 100]), ("x3f", [64, bc * 9]),
        ("x4t", [128, bc]), ("x5t", [10, bc]),
        ("wt_p0", [128, 36]), ("wt_g0", [128, 9 * 128]),
        ("wt_p1", [128, 9 * 128]), ("wt_a2", [128, 9 * 64]),
        ("wt_p2", [128, 4 * 9 * 64]), ("wt_a3", [128, 4 * 9 * 64]),
        ("wt_p3", [128, 9 * 64]), ("wt_a4", [64, 9 * 128]),
        ("wt_p4", [10, 128]), ("wt_a5", [128, 10]),
        ("b1r", [128, 1]), ("b2r", [128, 1]), ("b3r", [64, 1]), ("b5r", [128, 1]),
        ("b4big", [64, bt * 9]),
    ]:
        dram[name] = nc.dram_tensor(name, shape, F32, kind="ExternalInput").ap()
    x5out = nc.dram_tensor("x5out", [10, bc], F32, kind="ExternalOutput").ap()

    g = GAMMA

    with tile.TileContext(nc) as tc:
        with (
            tc.tile_pool(name="state", bufs=1) as st,
            tc.tile_pool(name="wts", bufs=1) as wp,
            tc.tile_pool(name="tmp", bufs=1) as tp,
            tc.tile_pool(name="pp", bufs=3, space=bass.MemorySpace.PSUM) as pp,
            tc.tile_pool(name="pg", bufs=4, space=bass.MemorySpace.PSUM) as pg,
            tc.tile_pool(name="pf", bufs=1, space=bass.MemorySpace.PSUM) as pf,
        ):
            # persistent state tiles. Conv inputs are stored width-padded
            # with 64 elements of trailing slack: matmul taps read flat
            # overlapping windows that may run up to 2 elements past the last
            # sample (junk output columns, skipped by consumers).
            x1t = st.tile([128, bt, 12, 12], F32)
            x1rpF = st.tile([128, bt * 256 + 64], F32)
            m1tF = st.tile([128, bt * 144 + 64], F32)
            x2t = st.tile([64, bt, 10, 10], F32)
            x2rpF = st.tile([128, bt * 196 + 64], F32)
            m2tF = st.tile([128, bt * 100 + 64], F32)
            x3t = st.tile([64, bt, 3, 3], F32)
            x3rpF = st.tile([128, bt * 49 + 64], F32)
            m3t = st.tile([64, bt, 9], F32)
            x4t = st.tile([128, bt], F32)
            x4r = st.tile([128, bt], F32)
            m4t = st.tile([128, bt], F32)
            x5t = st.tile([10, bt], F32)
            x5r = st.tile([10, bt], F32)
            obst = st.tile([128, 8, 196], F32)
            m0tF = st.tile([128, 8 * 196 + 64], F32)

            def view4(flat, n, d1, d2):
                return flat[:, 0:bt * n].rearrange(
                    "p (b h w) -> p b h w", h=d1, w=d2)

            def view5(flat, n, q, d1, d2):
                return flat[:, 0:bt * n].rearrange(
                    "p (b q h w) -> p b q h w", q=q, h=d1, w=d2)

            x1rp = view4(x1rpF, 256, 16, 16)
            m1t = view4(m1tF, 144, 12, 12)
            x2rp = view4(x2rpF[0:64, :], 196, 14, 14)
            m2t = view5(m2tF[0:64, :], 100, 4, 5, 5)
            x3rp = view4(x3rpF[0:64, :], 49, 7, 7)
            m0tq = [m0tF[32 * q:32 * q + 4, 0:8 * 196].rearrange(
                "p (b h w) -> p b h w", h=14, w=14) for q in range(4)]

            def rawview(ap_full, dims):
                pp = ap_full.ap.to_list()[0]
                return bass.AP(ap_full.tensor, ap_full.offset, [pp] + dims)

            def winapq(sl, sstride, b0, nb, off, wlen):
                """winap on a partition-sliced flat view (e.g. one quarter
                of m0tF at partitions 32q..32q+4)."""
                pp = sl.ap.to_list()[0]
                return bass.AP(sl.tensor, sl.offset + b0 * sstride + off,
                               [pp, [sstride, nb], [1, wlen]]).bitcast(F32R)

            def winap(flat, sstride, b0, nb, off, wlen):
                """Overlapping flat window [nb (stride sstride), wlen] at
                offset b0*sstride+off — a matmul-legal 2-free-dim AP."""
                pp = flat.ap.to_list()[0]
                return bass.AP(flat.tensor, b0 * sstride + off,
                               [pp, [sstride, nb], [1, wlen]]).bitcast(F32R)

            # weights
            wt = {}
            _R_WTS = {"wt_p0", "wt_g0", "wt_p1", "wt_a2", "wt_p2", "wt_a3"}
            for name in ["wt_p0", "wt_g0", "wt_p1", "wt_a2", "wt_p2", "wt_a3",
                         "wt_p3", "wt_a4", "wt_p4", "wt_a5",
                         "b1r", "b2r", "b3r", "b5r", "b4big"]:
                dt = F32R if name in _R_WTS else F32
                wt[name] = wp.tile(list(dram[name].shape), dt, name=f"w_{name}",
                                   tag=name)
                nc.sync.dma_start(wt[name][:], dram[name][:].bitcast(dt))

            def mm(ps_ap, lhsT, rhs, first, last, rdt=F32R, tp_=None):
                if lhsT.dtype != rdt:
                    lhsT = lhsT.bitcast(rdt)
                if rhs.dtype != rdt:
                    rhs = rhs.bitcast(rdt)
                nc.tensor.matmul(ps_ap, lhsT, rhs, start=first, stop=last,
                                 tile_position=tp_)

            def step_body():
                # ---- small relus ----
                _roots = [
                    nc.vector.tensor_scalar_max(x4r[:], x4t[:], 0.0),
                    nc.vector.tensor_scalar_max(x5r[:], x5t[:], 0.0),
                    nc.vector.tensor_scalar_max(
                        x3rp[:, :, 2:5, 2:5].bitcast(F32R), x3t[:], 0.0),
                    nc.vector.tensor_scalar_max(
                        x1rp[:, :, 2:14, 2:14].bitcast(F32R), x1t[:], 0.0),
                    nc.vector.tensor_scalar_max(
                        x2rp[:, :, 2:12, 2:12].bitcast(F32R), x2t[:], 0.0),
                ]

                # mirror relu'd conv inputs onto PE rows 64-127 so p1/p2
                # can run tap/parity halves on both array strips concurrently
                nc.sync.dma_start(x2rpF[64:128, :], x2rpF[0:64, :])
                nc.sync.dma_start(x3rpF[64:128, :], x3rpF[0:64, :])

                # ---- p2 (2 concurrent parity pairs) + m2 ----
                def p2_consume(pq, ps, b0, nb):
                    py, px = pq // 2, pq % 2
                    xsl = x2t[:, b0:b0 + nb, py::2, px::2]
                    psv = rawview(ps[:], [[33, nb], [7, 5], [1, 5]])
                    # 1-input op tolerates the 4D psum view; the 2-input
                    # stt below then sees only <=3D operands.
                    cmp2 = tp.tile([64, P2NB * 25], F32, tag="m2c",
                                   name=f"cmp2_{pq}")
                    nc.vector.tensor_scalar(cmp2[:, :nb * 25], psv,
                                            wt["b3r"][:, 0:1], 0.0,
                                            Op.add, Op.add)
                    nc.vector.scalar_tensor_tensor(
                        m2t[:, b0:b0 + nb, pq].bitcast(F32R),
                        cmp2[:, :nb * 25], 0.0, xsl,
                        Op.add, Op.subtract)

                for pq in range(2):
                    for (b0, nb) in _chunks(bt, P2NB):
                        psA = pp.tile([64, P2NB, 33], F32, tag="pred",
                                      name="p2A")
                        psB = pp.tile([64, P2NB, 33], F32, tag="pred",
                                      name="p2B")
                        sslA = psA[:, :nb].rearrange("p b w -> p (b w)")
                        sslB = psB[:, :nb].rearrange("p b w -> p (b w)")
                        for t9 in range(9):
                            ti, tj = t9 // 3, t9 % 3
                            rhsA = winapq(x3rpF[0:64, :], 49, b0, nb, ti * 7 + tj, 33)
                            mm(sslA, wt["wt_p2"][0:64, (pq * 9 + t9) * 64:
                                                 (pq * 9 + t9 + 1) * 64],
                               rhsA, t9 == 0, t9 == 8)
                            rhsB = winapq(x3rpF[64:128, :], 49, b0, nb,
                                          ti * 7 + tj, 33)
                            mm(sslB, wt["wt_p2"][64:128,
                                                 ((pq + 2) * 9 + t9) * 64:
                                                 ((pq + 2) * 9 + t9 + 1) * 64],
                               rhsB, t9 == 0, t9 == 8)
                        p2_consume(pq, psA, b0, nb)
                        p2_consume(pq + 2, psB, b0, nb)

                # ---- p3 + m3 ----
                ps3 = pf.tile([64, bt, 9], F32, tag="fc")
                for s in range(9):
                    lhsT = wt["wt_p3"][:, s * 64:(s + 1) * 64]
                    mm(ps3[:, :, s], lhsT, x4r[:], True, True, rdt=F32)
                nc.vector.tensor_tensor(m3t[:], ps3[:], x3t[:], Op.subtract)
                nc.vector.tensor_tensor(m3t[:], m3t[:], wt["b4big"][:], Op.add)

                # ---- p4 + m4 ----
                ps4 = pf.tile([128, bt], F32, tag="fc")
                mm(ps4[:], wt["wt_p4"][:], x5r[:], True, True, rdt=F32)
                nc.vector.scalar_tensor_tensor(
                    m4t[:], ps4[:], wt["b5r"][:, 0:1], x4t[:], Op.add, Op.subtract)

                # ---- p0 + m0 (quarters on col strips 32q, concurrent) ----
                for (b0, nb) in _chunks(8, P0NB):
                    ps = pp.tile([128, P0NB, 14, 16], F32, tag="pred")
                    for t9 in range(9):
                        ti, tj = t9 // 3, t9 % 3
                        lhsT = wt["wt_p0"][:, t9 * 4:(t9 + 1) * 4]
                        for q in range(4):
                            pssl = ps[32 * q:32 * q + 4, :nb].rearrange(
                                "p b h w -> p (b h w)")
                            rhs = winap(x1rpF, 256, q * 8 + b0, nb,
                                        ti * 16 + tj, 224)
                            mm(pssl, lhsT, rhs, t9 == 0, t9 == 8,
                               tp_=(0, 32 * q))
                    for q in range(4):
                        nc.vector.scalar_tensor_tensor(
                            m0tq[q][:, b0:b0 + nb].bitcast(F32R),
                            ps[32 * q:32 * q + 4, :nb, :, 0:14],
                            wt["b1r"][32 * q:32 * q + 4, 0:1],
                            obst[32 * q:32 * q + 4, b0:b0 + nb], Op.add,
                            Op.subtract)

                # ---- p1 + m1 ----
                for (b0, nb) in _chunks(bt, P1NB):
                    psA = pp.tile([128, P1NB, 12, 14], F32, tag="pred",
                                  name="p1A")
                    psB = pp.tile([128, P1NB, 12, 14], F32, tag="pred",
                                  name="p1B")
                    sslA = psA[:, :nb].rearrange("p b h w -> p (b h w)")
                    sslB = psB[:, :nb].rearrange("p b h w -> p (b h w)")
                    seq = []
                    for k in range(5):
                        seq.append((0, k))
                        if k < 4:
                            seq.append((1, 5 + k))
                    for (up, t9) in seq:
                        ti, tj = t9 // 3, t9 % 3
                        if not up:
                            rhs = winapq(x2rpF[0:64, :], 196, b0, nb, ti * 14 + tj, 168)
                            mm(sslA, wt["wt_p1"][0:64,
                                                 t9 * 128:(t9 + 1) * 128],
                               rhs, t9 == 0, t9 == 4)
                        else:
                            rhs = winapq(x2rpF[64:128, :], 196, b0, nb,
                                         ti * 14 + tj, 168)
                            mm(sslB, wt["wt_p1"][64:128,
                                                 t9 * 128:(t9 + 1) * 128],
                               rhs, t9 == 5, t9 == 8)
                    # DVE may read only one PSUM operand per op: build m1
                    # from bank A via the stt, then add bank B separately.
                    nc.vector.scalar_tensor_tensor(
                        m1t[:, b0:b0 + nb].bitcast(F32R), psA[:, :nb, :, 0:12],
                        wt["b2r"][:, 0:1], x1t[:, b0:b0 + nb], Op.add,
                        Op.subtract)
                    nc.vector.tensor_tensor(
                        m1t[:, b0:b0 + nb].bitcast(F32R),
                        m1t[:, b0:b0 + nb], psB[:, :nb, :, 0:12], Op.add)

                # ---- G0 + x1 update (quarters on row strips 32q) ----
                for (b0, nb) in _chunks(8, G0NB):
                    psq = [pg.tile([128, G0NB, 12, 14], F32, tag="grad",
                                   name=f"psg{q}") for q in range(4)]
                    for t9 in range(9):
                        ti, tj = t9 // 3, t9 % 3
                        for q in range(4):
                            pssl = psq[q][:, :nb].rearrange(
                                "p b h w -> p (b h w)")
                            rhs = winapq(m0tF[32 * q:32 * q + 4, :], 196, b0,
                                         nb, ti * 14 + tj, 168)
                            lhsT = wt["wt_g0"][32 * q:32 * q + 4,
                                               t9 * 128:(t9 + 1) * 128]
                            mm(pssl, lhsT, rhs, t9 == 0, t9 == 8,
                               tp_=(32 * q, 0))
                    for q in range(4):
                        s0 = q * 8 + b0
                        u = tp.tile([128, G0NB * 144], F32, tag="u1")
                        usl = u[:, : nb * 144]
                        xsl = x1t[:, s0:s0 + nb]
                        nc.vector.scalar_tensor_tensor(usl, xsl, 0.0,
                                                       psq[q][:, :nb, :, 0:12],
                                                       Op.is_gt, Op.mult)
                        nc.vector.scalar_tensor_tensor(xsl, usl, g, xsl,
                                                       Op.mult, Op.add)
                        nc.vector.scalar_tensor_tensor(xsl, m1t[:, s0:s0 + nb],
                                                       g, xsl, Op.mult, Op.add)

                # ---- A2 + x2 update (drelu part) ----
                for (b0, nb) in _chunks(bt, A2NB):
                    ps = pg.tile([64, A2NB, 10, 12], F32, tag="grad")
                    pssl = ps[:, :nb].rearrange("p b h w -> p (b h w)")
                    for t9 in range(9):
                        ti, tj = t9 // 3, t9 % 3
                        rhs = winap(m1tF, 144, b0, nb, ti * 12 + tj, 120)
                        lhsT = wt["wt_a2"][:, t9 * 64:(t9 + 1) * 64]
                        mm(pssl, lhsT, rhs, t9 == 0, t9 == 8)
                    u = tp.tile([64, A2NB * 100], F32, tag="u2")
                    usl = u[:, : nb * 100]
                    xsl = x2t[:, b0:b0 + nb]
                    nc.vector.scalar_tensor_tensor(usl, xsl, 0.0,
                                                   ps[:, :nb, :, 0:10],
                                                   Op.is_gt, Op.mult)
                    nc.vector.scalar_tensor_tensor(xsl, usl, g, xsl,
                                                   Op.mult, Op.add)
                # x2 += g * m2 (parity-strided)
                for pq in range(4):
                    py, px = pq // 2, pq % 2
                    xsl = x2t[:, :, py::2, px::2]
                    nc.vector.scalar_tensor_tensor(xsl, m2t[:, :, pq], g, xsl,
                                                   Op.mult, Op.add)

                # ---- A3 + x3 update (parity pairs on both strips) ----
                nc.sync.dma_start(m2tF[64:128, :], m2tF[0:64, :])
                psa3 = pg.tile([64, bt, 13], F32, tag="grad", name="a3A")
                psa3B = pg.tile([64, bt, 13], F32, tag="grad", name="a3B")
                psa3f = psa3[:].rearrange("p b w -> p (b w)")
                psa3fB = psa3B[:].rearrange("p b w -> p (b w)")
                for idx in range(18):
                    pq, t9 = divmod(idx, 9)
                    ti, tj = t9 // 3, t9 % 3
                    rhs = winapq(m2tF[0:64, :], 100, 0, bt, pq * 25 + ti * 5 + tj, 13)
                    lhsT = wt["wt_a3"][0:64,
                                       (pq * 9 + t9) * 64:(pq * 9 + t9 + 1) * 64]
                    mm(psa3f, lhsT, rhs, idx == 0, idx == 17)
                    pqB = pq + 2
                    rhsB = winapq(m2tF[64:128, :], 100, 0, bt,
                                  pqB * 25 + ti * 5 + tj, 13)
                    lhsTB = wt["wt_a3"][64:128,
                                        (pqB * 9 + t9) * 64:(pqB * 9 + t9 + 1) * 64]
                    mm(psa3fB, lhsTB, rhsB, idx == 0, idx == 17)
                u3 = tp.tile([64, bt * 9], F32, tag="u3")
                psa3v = rawview(psa3[:], [[13, bt], [5, 3], [1, 3]])
                cmp3 = tp.tile([64, bt * 9], F32, tag="m3c")
                nc.vector.tensor_scalar(cmp3[:], psa3v, 0.0, 0.0, Op.add, Op.add)
                psa3Bv = rawview(psa3B[:], [[13, bt], [5, 3], [1, 3]])
                nc.vector.tensor_tensor(cmp3[:], cmp3[:], psa3Bv, Op.add)
                nc.vector.scalar_tensor_tensor(u3[:], x3t[:], 0.0, cmp3[:],
                                               Op.is_gt, Op.mult)
                nc.vector.scalar_tensor_tensor(x3t[:], u3[:], g, x3t[:],
                                               Op.mult, Op.add)
                nc.vector.scalar_tensor_tensor(x3t[:], m3t[:], g, x3t[:],
                                               Op.mult, Op.add)

                # ---- A4 + x4 update ----
                ps4g = pf.tile([128, bt], F32, tag="fc")
                for s in range(9):
                    lhsT = wt["wt_a4"][:, s * 128:(s + 1) * 128]
                    rhs = m3t[:, :, s]
                    mm(ps4g[:], lhsT, rhs, s == 0, s == 8, rdt=F32)
                u4 = tp.tile([128, bt], F32, tag="u4")
                nc.vector.scalar_tensor_tensor(u4[:], x4t[:], 0.0, ps4g[:],
                                               Op.is_gt, Op.mult)
                nc.vector.scalar_tensor_tensor(x4t[:], u4[:], g, x4t[:],
                                               Op.mult, Op.add)
                nc.vector.scalar_tensor_tensor(x4t[:], m4t[:], g, x4t[:],
                                               Op.mult, Op.add)

                # ---- A5 + x5 update ----
                ps5 = pf.tile([10, bt], F32, tag="fc")
                mm(ps5[:], wt["wt_a5"][:], m4t[:], True, True, rdt=F32)
                u5 = tp.tile([10, bt], F32, tag="u5")
                nc.vector.scalar_tensor_tensor(u5[:], x5t[:], 0.0, ps5[:],
                                               Op.is_gt, Op.mult)
                nc.vector.scalar_tensor_tensor(x5t[:], u5[:], g, x5t[:],
                                               Op.mult, Op.add)
                return _roots

            def chunk_body(c):
                nc.sync.dma_start(x1t[:], dram["x1p"][:, c * bt * 144:(c + 1) * bt * 144])
                for q in range(4):
                    nc.sync.dma_start(
                        obst[32 * q:32 * q + 4].rearrange("p b s -> p (b s)"),
                        dram["obsp"][4 * q:4 * q + 4,
                                     c * 8 * 196:(c + 1) * 8 * 196])
                nc.sync.dma_start(x2t[:], dram["x2f"][:, c * bt * 100:(c + 1) * bt * 100])
                nc.sync.dma_start(x3t[:], dram["x3f"][:, c * bt * 9:(c + 1) * bt * 9])
                nc.sync.dma_start(x4t[:], dram["x4t"][:, c * bt:(c + 1) * bt])
                nc.sync.dma_start(x5t[:], dram["x5t"][:, c * bt:(c + 1) * bt])
                absorb = tp.tile([10, 1], F32, name="absorb", tag="absorb",
                                 bufs=2)
                ab = nc.vector.tensor_scalar_add(absorb[:], x5t[:, 0:1], 0.0)
                roots = None
                if steps > 1 and not unroll_steps:
                    from concourse.engine_type import EngineType as _ET
                    with tc.For_i(0, steps, 1,
                                  hint_engines=(_ET.PE, _ET.DVE, _ET.SP)):
                        for _ in range(1):
                            r = step_body()
                            if roots is None:
                                roots = r
                else:
                    for _ in range(steps):
                        r = step_body()
                        if roots is None:
                            roots = r
                from concourse.tile import add_dep_helper
                for r in roots:
                    add_dep_helper(r.ins, ab.ins, sync=False,
                                   reason="absorb DMA waits on DVE first")
                x5cpy = tp.tile([10, bt], F32, name="x5cpy", tag="x5cpy",
                                bufs=2)
                nc.vector.tensor_copy(x5cpy[:], x5t[:])
                nc.sync.dma_start(x5out[:, c * bt:(c + 1) * bt], x5cpy[:])

            # zero the padded conv-input tiles once (borders/slack persist;
            # interiors are rewritten each step).  (x is_gt inf) == 0.0 even
            # for uninitialized/NaN input, and the output is f32r-tagged for
            # the BIR verifier.
            for _zt in (x1rpF, m1tF, x2rpF, m2tF, x3rpF, m0tF):
                nc.vector.memset(_zt[:], 0.0)
                nc.vector.tensor_scalar(_zt[:].bitcast(F32R), _zt[:],
                                        3e38, 0.0, Op.is_gt, Op.mult)
            for c in range(nchunk):
                chunk_body(c)

    _split_multiwait_instructions(nc)
    nc.finalize()
    return nc


# ---------------- public entry point ----------------

def kernel(obs, x1, x2, x3, x4, x5, W1, b1, W2, b2, W3, b3, W4, b4, W5, b5):
    from concourse.bass_utils import run_bass_kernel_spmd

    obs, x1, x2, x3, x4, x5 = [np.asarray(a, np.float32)
                               for a in (obs, x1, x2, x3, x4, x5)]
    w = make_weight_arrays(np.asarray(W1), np.asarray(b1), np.asarray(W2),
                           np.asarray(b2), np.asarray(W3), np.asarray(b3),
                           np.asarray(W4), np.asarray(b4), np.asarray(W5),
                           np.asarray(b5), BT)
    in_maps = []
    for c in range(NCORES):
        sl = slice(c * BC, (c + 1) * BC)
        d = make_data_arrays(obs[sl], x1[sl], x2[sl], x3[sl], x4[sl], x5[sl])
        d.update(w)
        in_maps.append(d)

    nc = build_bass()
    res = run_bass_kernel_spmd(nc, in_maps, core_ids=list(range(NCORES)))
    out = np.empty((B, 10), np.float32)
    for c in range(NCORES):
        out[c * BC:(c + 1) * BC] = res.results[c]["x5out"].T
    return out

